# revision 13
# baseline (speedup 1.0000x reference)
"""GroupedLinear Trainium2 kernel (v2: bf16 + fp8-DoubleRow hybrid).

Math: out[b, g*R + r] = sum_s x[b, perm[g, s]] * W[g, r, s] + bias[g, r]
with B=8192, C=4096, G=16, S=256, R=512.

Strategy (batch-parallel over 8 cores, BC=1024 columns/core):
* Host: apply the channel permutation while building a transposed,
  group-contiguous activation tensor x[(g,k,s'), b] and per-group transposed
  weights W[g][s', k, r]. A subset FP8_GROUPS of the 16 groups is shipped as
  fp8e4m3 (both operands); the rest bf16. Bias is folded in on the host.
* Device, per group:
  - bf16 group: per r-tile rt (4) and batch half bh (2): 2 K=128 bf16
    matmuls accumulate into one half of a 2-bank PSUM tile [128, 2, 512].
  - fp8 group: per (rt, bh): ONE DoubleRow fp8 matmul (K=256: lhsT
    [128,2,128], rhs [128,2,512]) into the same PSUM shape. DoubleRow is 2x
    on the PE; the e4m3 quantization error is rel-L2 3.8e-2 for a full-fp8
    group, so only 4/16 groups run fp8 keeping total rel err ~1.9e-2 < 2e-2.
  - copies: one [128, 1024] fp32->bf16 copy per r-tile (reads both PSUM
    banks), alternating ACT / DVE.
  - loads: one x DMA + one w DMA per group (SP for x, Pool SWDGE for w),
    except group 0 which keeps the split load order that gets the first
    matmul started at ~2.4us (DMA latency floor).
  - stores: one [128, 4, 1024] DMA per group into a transposed output
    outT2[p, g*4+rt, b] (host untransposes), alternating SP/Pool. The last
    group is hand-scheduled: per-bh copies, per-rt stores, and the final
    r-tile's second half is split 384+128 so the very last copy+store
    chain (the unavoidable ~2.7us DMA-drain tail) carries only 32KB.

Notes:
* PE roofline (bf16, all 16 groups) is 54.6us/core; baseline measured
  59.8us. fp8 DoubleRow on 4 groups cuts PE work by 12.5% (hw 2x).
* DMA dispatch time lands on the issuing engine (~0.65-1.0us per DMA), so
  coarse per-group DMAs keep SP/Pool far below the PE time.
* fp8 DoubleRow with full 128-partition output works on hw (verified
  against numpy at rel 1e-4); the earlier "PSUM partitions 0-63" note was
  wrong.
* The PE p-state ramp is wall-clock based (full speed after 3us); the ~2
  matmuls that run before 3us at half clock are unavoidable (DMA latency
  floor for the first operands).
"""

import numpy as np
import ml_dtypes

import concourse.bass as bass
import concourse.mybir as mybir
import concourse.tile as tile
from concourse import bacc
from concourse.bass_utils import run_bass_kernel_spmd

B, C, G, S, R = 8192, 4096, 16, 256, 512
N_CORES = 8
BC = B // N_CORES          # 1024 batch columns per core
KCH = S // 128             # 2 contraction chunks per group
RT = R // 128              # 4 r-tiles per group
NB = 512                   # matmul moving-dim size (one PSUM bank of fp32)
BH = BC // NB              # 2 batch halves per core

# Which logical groups run in fp8 DoubleRow. Keep group 0 (tuned startup
# order) and group 15 (hand-scheduled tail) in bf16. Error budget: rel-L2
# grows as 3.8e-2 * sqrt(n/16); n=4 -> ~1.9e-2 (gate 2e-2).
FP8_GROUPS = (1, 4, 7, 10)

BF16 = mybir.dt.bfloat16
FP8 = mybir.dt.float8e4
F32 = mybir.dt.float32

_BASS_CACHE: dict = {}


def _build_bass():
    n8 = len(FP8_GROUPS)
    nb16 = G - n8
    fidx = {g: i for i, g in enumerate(FP8_GROUPS)}
    bidx = {g: i for i, g in enumerate(g for g in range(G) if g not in fidx)}

    nc = bacc.Bacc(None, num_swdge_queues=4)
    xb = nc.declare_dram_parameter("xb", [nb16, 128, KCH, BC], BF16, isOutput=False)
    wb = nc.declare_dram_parameter("wb", [nb16, 128, KCH, R], BF16, isOutput=False)
    if n8:
        x8 = nc.declare_dram_parameter("x8", [n8, 128, KCH, BC], FP8, isOutput=False)
        w8 = nc.declare_dram_parameter("w8", [n8, 128, KCH, R], FP8, isOutput=False)
    outT2 = nc.declare_dram_parameter("outT2", [128, G * RT, BC], BF16, isOutput=True)

    DR = mybir.MatmulPerfMode.DoubleRow

    # Store-engine schedule for the 30 half-group stores (groups 0-14, two
    # [128, 2, 1024] halves each). CoreSim charges DMA transfer time on the
    # issuing engine (~0.3855 ns per per-partition byte, 500ns floor), so
    # engine busy must be balanced: loads go x->SP, w->Pool; stores fill the
    # remaining SP/Pool capacity only -- a store on ACT/DVE would delay
    # pending PSUM copies, which stalls the PE via the 4-deep PSUM rotation.
    store_share = {"gpsimd": 18, "sync": 12}
    store_seq = []
    used = dict.fromkeys(store_share, 0)
    for i in range(30):
        eng = max(store_share,
                  key=lambda e: store_share[e] * (i + 1) / 30 - used[e])
        used[eng] += 1
        store_seq.append(eng)

    with tile.TileContext(nc) as tc:
        with (
            tc.tile_pool(name="xbp", bufs=4) as xbp,
            tc.tile_pool(name="wbp", bufs=4) as wbp,
            tc.tile_pool(name="x8p", bufs=2) as x8p,
            tc.tile_pool(name="w8p", bufs=2) as w8p,
            tc.tile_pool(name="op", bufs=4) as op,
            tc.tile_pool(name="pp", bufs=4, space="PSUM") as pp,
        ):
            copy_flip = 0
            store_i = 0
            engines = {"sync": nc.sync, "gpsimd": nc.gpsimd,
                       "scalar": nc.scalar}

            def copy_eng():
                nonlocal copy_flip
                copy_flip += 1
                return nc.scalar if copy_flip % 2 else nc.vector

            def store_eng():
                nonlocal store_i
                eng = engines[store_seq[store_i]]
                store_i += 1
                return eng

            def emit_loads(g):
                is8 = g in fidx
                if is8:
                    xg = x8p.tile([128, KCH, BC], FP8, tag="x8")
                    wg = w8p.tile([128, KCH, R], FP8, tag="w8")
                    nc.sync.dma_start(out=xg[:], in_=x8[fidx[g]])
                    nc.gpsimd.dma_start(out=wg[:], in_=w8[fidx[g]])
                else:
                    xg = xbp.tile([128, KCH, BC], BF16, tag="xb")
                    wg = wbp.tile([128, KCH, R], BF16, tag="wb")
                    if g == 0:
                        # tuned startup: the first matmul's operands (w k0 on
                        # SP, x k0 first half on Pool) land as early as
                        # possible
                        nc.sync.dma_start(out=wg[:, 0, :], in_=wb[0][:, 0, :])
                        nc.gpsimd.dma_start(out=xg[:, 0, :NB], in_=xb[0][:, 0, :NB])
                        nc.gpsimd.dma_start(out=xg[:, 0, NB:], in_=xb[0][:, 0, NB:])
                        nc.gpsimd.dma_start(out=xg[:, 1, :], in_=xb[0][:, 1, :])
                        nc.sync.dma_start(out=wg[:, 1, :], in_=wb[0][:, 1, :])
                    else:
                        nc.sync.dma_start(out=xg[:], in_=xb[bidx[g]])
                        nc.gpsimd.dma_start(out=wg[:], in_=wb[bidx[g]])
                og = op.tile([128, RT, BC], BF16, tag="o")
                return {"is8": is8, "xg": xg, "wg": wg, "og": og}

            def mm(st, ps_dst, rt, b0, b1):
                """Accumulate x[:, b0:b1] @ W[rt-tile] into ps_dst."""
                if st["is8"]:
                    nc.tensor.matmul(
                        out=ps_dst,
                        lhsT=st["wg"][:, :, rt * 128:(rt + 1) * 128],
                        rhs=st["xg"][:, :, b0:b1],
                        start=True, stop=True, perf_mode=DR,
                    )
                else:
                    for k in range(KCH):
                        nc.tensor.matmul(
                            out=ps_dst,
                            lhsT=st["wg"][:, k, rt * 128:(rt + 1) * 128],
                            rhs=st["xg"][:, k, b0:b1],
                            start=(k == 0), stop=(k == KCH - 1),
                        )

            def do_tile(g, st, rt):
                ps = pp.tile([128, BH, NB], F32, tag="ps")
                for bh in range(BH):
                    mm(st, ps[:, bh, :], rt, bh * NB, (bh + 1) * NB)
                og = st["og"]
                eng = copy_eng()
                if eng is nc.scalar:
                    eng.copy(out=og[:, rt, :], in_=ps[:])
                else:
                    eng.tensor_copy(out=og[:, rt, :], in_=ps[:])
                if rt % 2 == 1:
                    # half-group store right after the rt1 / rt3 copies
                    h = rt // 2
                    store_eng().dma_start(
                        out=outT2[:, g * RT + 2 * h:g * RT + 2 * h + 2, :],
                        in_=og[:, 2 * h:2 * h + 2, :])

            def do_tail_group(g, st):
                # Hand-scheduled tail group: per-rt stores, per-bh copies for
                # rt2, and rt3's second half split 384+128 so the final
                # copy+store chain carries only 32KB.
                og = st["og"]
                for rt in range(RT - 1):
                    ps = pp.tile([128, BH, NB], F32, tag="ps")
                    for bh in range(BH):
                        mm(st, ps[:, bh, :], rt, bh * NB, (bh + 1) * NB)
                        if rt == RT - 2:
                            dst = og[:, rt, bh * NB:(bh + 1) * NB]
                            if bh == 0:
                                nc.scalar.copy(out=dst, in_=ps[:, bh, :])
                            else:
                                nc.vector.tensor_copy(out=dst, in_=ps[:, bh, :])
                    if rt != RT - 2:
                        eng = copy_eng()
                        if eng is nc.scalar:
                            eng.copy(out=og[:, rt, :], in_=ps[:])
                        else:
                            eng.tensor_copy(out=og[:, rt, :], in_=ps[:])
                    seng = (nc.gpsimd, nc.sync, nc.gpsimd)[rt]
                    seng.dma_start(out=outT2[:, g * RT + rt, :],
                                   in_=og[:, rt, :])
                rt = RT - 1
                ps = pp.tile([128, BH, NB], F32, tag="ps")
                mm(st, ps[:, 0, :], rt, 0, NB)
                nc.scalar.copy(out=og[:, rt, :NB], in_=ps[:, 0, :])
                nc.sync.dma_start(out=outT2[:, g * RT + rt, :NB],
                                  in_=og[:, rt, :NB])
                # rt3's second half as two 256-col pieces, each in its own
                # PSUM bank with its own copy+store on engines that are idle
                # by then; whichever the scheduler runs last, the trailing
                # chain (sem + ~395 copy + 500 store + 1716 flight + barrier)
                # carries only 64KB.
                HP = (BC - NB) // 2
                mm(st, ps[:, 1, :HP], rt, NB, NB + HP)
                nc.vector.tensor_copy(out=og[:, rt, NB:NB + HP],
                                      in_=ps[:, 1, :HP])
                nc.sync.dma_start(
                    out=outT2[:, g * RT + rt, NB:NB + HP],
                    in_=og[:, rt, NB:NB + HP])
                ps2 = pp.tile([128, BH, NB], F32, tag="ps")
                mm(st, ps2[:, 0, :HP], rt, NB + HP, BC)
                nc.scalar.copy(out=og[:, rt, NB + HP:],
                               in_=ps2[:, 0, :HP])
                nc.scalar.dma_start(
                    out=outT2[:, g * RT + rt, NB + HP:],
                    in_=og[:, rt, NB + HP:])

            # Interleave each fp8 group's r-tiles with the following bf16
            # group: the PE produces PSUM tiles 4x faster (sim) during an fp8
            # group than the two copy engines drain them, so alternating
            # bf16/fp8 tiles keeps the 4-deep PSUM rotation from stalling.
            g = 0
            while g < G:
                if g in fidx and g + 1 < G - 1:
                    stf = emit_loads(g)
                    stb = emit_loads(g + 1)
                    for rt in range(RT):
                        do_tile(g, stf, rt)
                        do_tile(g + 1, stb, rt)
                    g += 2
                else:
                    st = emit_loads(g)
                    if g == G - 1:
                        do_tail_group(g, st)
                    else:
                        for rt in range(RT):
                            do_tile(g, st, rt)
                    g += 1
    if not nc.is_finalized():
        nc.finalize()
    return nc


def _get_bass():
    if "nc" not in _BASS_CACHE:
        _BASS_CACHE["nc"] = _build_bass()
    return _BASS_CACHE["nc"]


def _prepare_inputs(x, W, b, perm):
    bf16 = ml_dtypes.bfloat16
    fp8 = ml_dtypes.float8_e4m3
    fset = set(FP8_GROUPS)
    perm_flat = np.asarray(perm).reshape(-1)

    # Gather the permuted channels (within-row gather: cache friendly), cast
    # to bf16, then transpose to channel-major [C, B] = [(g,k,s'), b].
    xg = np.ascontiguousarray(x)[:, perm_flat].astype(bf16)   # [B, C]
    xT = np.ascontiguousarray(xg.T).reshape(G, KCH, 128, B)   # [g, k, s', b]

    # weights: Wt[g][s', k, r] = W[g, r, k*128+s']
    Wt = np.asarray(W).reshape(G, R, KCH, 128).transpose(0, 3, 2, 1)  # [g,s',k,r]
    Wt = np.ascontiguousarray(Wt).astype(bf16)

    bgs = [g for g in range(G) if g not in fset]
    in_maps = []
    wb_all = np.stack([Wt[g] for g in bgs]) if bgs else None
    w8_all = (np.stack([Wt[g].astype(fp8) for g in FP8_GROUPS])
              if FP8_GROUPS else None)
    for c in range(N_CORES):
        sl = slice(c * BC, (c + 1) * BC)
        # [g, k, s', bc] -> per-group [s', k, bc]
        xc = xT[:, :, :, sl].transpose(0, 2, 1, 3)            # [g, s', k, bc]
        m = {
            "xb": np.ascontiguousarray(np.stack([xc[g] for g in bgs])),
            "wb": wb_all,
        }
        if FP8_GROUPS:
            m["x8"] = np.ascontiguousarray(
                np.stack([xc[g] for g in FP8_GROUPS])).astype(fp8)
            m["w8"] = w8_all
        in_maps.append(m)
    return in_maps


def kernel(x, W, b, perm, _trace=False, _trace_kwargs=None):
    nc = _get_bass()
    in_maps = _prepare_inputs(x, W, b, perm)
    res = run_bass_kernel_spmd(
        nc, in_maps, list(range(N_CORES)),
        trace=_trace, **(_trace_kwargs or {}),
    )
    b_flat = np.asarray(b, dtype=np.float32).reshape(-1)
    out = np.empty((B, G * R), dtype=np.float32)
    for c in range(N_CORES):
        blk = res.results[c]["outT2"]                    # [128, G*RT, BC] bf16
        blk = np.ascontiguousarray(blk.transpose(1, 0, 2)).reshape(G * R, BC)
        blk = blk.T.astype(np.float32)                   # [BC, G*R]
        blk += b_flat[None, :]
        out[c * BC:(c + 1) * BC, :] = blk
    if _trace:
        return out, res
    return out


# revision 14
# speedup vs baseline: 1.0081x; 1.0081x over previous
"""GroupedLinear Trainium2 kernel (v2: bf16 + fp8-DoubleRow hybrid).

Math: out[b, g*R + r] = sum_s x[b, perm[g, s]] * W[g, r, s] + bias[g, r]
with B=8192, C=4096, G=16, S=256, R=512.

Strategy (batch-parallel over 8 cores, BC=1024 columns/core):
* Host: apply the channel permutation while building a transposed,
  group-contiguous activation tensor x[(g,k,s'), b] and per-group transposed
  weights W[g][s', k, r]. A subset FP8_GROUPS of the 16 groups is shipped as
  fp8e4m3 (both operands); the rest bf16. Bias is folded in on the host.
* Device, per group:
  - bf16 group: per r-tile rt (4) and batch half bh (2): 2 K=128 bf16
    matmuls accumulate into one half of a 2-bank PSUM tile [128, 2, 512].
  - fp8 group: per (rt, bh): ONE DoubleRow fp8 matmul (K=256: lhsT
    [128,2,128], rhs [128,2,512]) into the same PSUM shape. DoubleRow is 2x
    on the PE; the e4m3 quantization error is rel-L2 3.8e-2 for a full-fp8
    group, so only 4/16 groups run fp8 keeping total rel err ~1.9e-2 < 2e-2.
  - copies: one [128, 1024] fp32->bf16 copy per r-tile (reads both PSUM
    banks), alternating ACT / DVE.
  - loads: one x DMA + one w DMA per group (SP for x, Pool SWDGE for w),
    except group 0 which keeps the split load order that gets the first
    matmul started at ~2.4us (DMA latency floor).
  - stores: one [128, 4, 1024] DMA per group into a transposed output
    outT2[p, g*4+rt, b] (host untransposes), alternating SP/Pool. The last
    group is hand-scheduled: per-bh copies, per-rt stores, and the final
    r-tile's second half is split 384+128 so the very last copy+store
    chain (the unavoidable ~2.7us DMA-drain tail) carries only 32KB.

Notes:
* PE roofline (bf16, all 16 groups) is 54.6us/core; baseline measured
  59.8us. fp8 DoubleRow on 4 groups cuts PE work by 12.5% (hw 2x).
* DMA dispatch time lands on the issuing engine (~0.65-1.0us per DMA), so
  coarse per-group DMAs keep SP/Pool far below the PE time.
* fp8 DoubleRow with full 128-partition output works on hw (verified
  against numpy at rel 1e-4); the earlier "PSUM partitions 0-63" note was
  wrong.
* The PE p-state ramp is wall-clock based (full speed after 3us); the ~2
  matmuls that run before 3us at half clock are unavoidable (DMA latency
  floor for the first operands).
"""

import numpy as np
import ml_dtypes

import concourse.bass as bass
import concourse.mybir as mybir
import concourse.tile as tile
from concourse import bacc
from concourse.bass_utils import run_bass_kernel_spmd

B, C, G, S, R = 8192, 4096, 16, 256, 512
N_CORES = 8
BC = B // N_CORES          # 1024 batch columns per core
KCH = S // 128             # 2 contraction chunks per group
RT = R // 128              # 4 r-tiles per group
NB = 512                   # matmul moving-dim size (one PSUM bank of fp32)
BH = BC // NB              # 2 batch halves per core

# Which logical groups run in fp8 DoubleRow. Keep group 0 (tuned startup
# order) and group 15 (hand-scheduled tail) in bf16. Error budget: rel-L2
# grows as 3.8e-2 * sqrt(n/16); n=4 -> ~1.9e-2 (gate 2e-2).
FP8_GROUPS = (1, 4, 7, 10)

BF16 = mybir.dt.bfloat16
FP8 = mybir.dt.float8e4
F32 = mybir.dt.float32

_BASS_CACHE: dict = {}


def _build_bass():
    n8 = len(FP8_GROUPS)
    nb16 = G - n8
    fidx = {g: i for i, g in enumerate(FP8_GROUPS)}
    bidx = {g: i for i, g in enumerate(g for g in range(G) if g not in fidx)}

    nc = bacc.Bacc(None, num_swdge_queues=4)
    xb = nc.declare_dram_parameter("xb", [nb16, 128, KCH, BC], BF16, isOutput=False)
    wb = nc.declare_dram_parameter("wb", [nb16, 128, KCH, R], BF16, isOutput=False)
    if n8:
        x8 = nc.declare_dram_parameter("x8", [n8, 128, KCH, BC], FP8, isOutput=False)
        w8 = nc.declare_dram_parameter("w8", [n8, 128, KCH, R], FP8, isOutput=False)
    outT2 = nc.declare_dram_parameter("outT2", [128, G * RT, BC], BF16, isOutput=True)

    DR = mybir.MatmulPerfMode.DoubleRow

    # Store-engine schedule for the 30 half-group stores (groups 0-14, two
    # [128, 2, 1024] halves each). CoreSim charges DMA transfer time on the
    # issuing engine (~0.3855 ns per per-partition byte, 500ns floor), so
    # engine busy must be balanced: loads go x->SP, w->Pool; stores fill the
    # remaining SP/Pool capacity only -- a store on ACT/DVE would delay
    # pending PSUM copies, which stalls the PE via the 4-deep PSUM rotation.
    store_share = {"gpsimd": 18, "sync": 12}
    store_seq = []
    used = dict.fromkeys(store_share, 0)
    for i in range(30):
        eng = max(store_share,
                  key=lambda e: store_share[e] * (i + 1) / 30 - used[e])
        used[eng] += 1
        store_seq.append(eng)

    with tile.TileContext(nc) as tc:
        with (
            tc.tile_pool(name="xbp", bufs=4) as xbp,
            tc.tile_pool(name="wbp", bufs=4) as wbp,
            tc.tile_pool(name="x8p", bufs=2) as x8p,
            tc.tile_pool(name="w8p", bufs=2) as w8p,
            tc.tile_pool(name="op", bufs=4) as op,
            tc.tile_pool(name="pp", bufs=4, space="PSUM") as pp,
        ):
            copy_flip = 0
            store_i = 0
            engines = {"sync": nc.sync, "gpsimd": nc.gpsimd,
                       "scalar": nc.scalar}

            def copy_eng():
                nonlocal copy_flip
                copy_flip += 1
                return nc.scalar if copy_flip % 2 else nc.vector

            def store_eng():
                nonlocal store_i
                eng = engines[store_seq[store_i]]
                store_i += 1
                return eng

            def emit_loads(g):
                is8 = g in fidx
                if is8:
                    xg = x8p.tile([128, KCH, BC], FP8, tag="x8")
                    wg = w8p.tile([128, KCH, R], FP8, tag="w8")
                    nc.sync.dma_start(out=xg[:], in_=x8[fidx[g]])
                    nc.gpsimd.dma_start(out=wg[:], in_=w8[fidx[g]])
                else:
                    xg = xbp.tile([128, KCH, BC], BF16, tag="xb")
                    wg = wbp.tile([128, KCH, R], BF16, tag="wb")
                    if g == 0:
                        # tuned startup: the first matmul's operands (w k0 on
                        # SP, x k0 first half on Pool) land as early as
                        # possible
                        nc.sync.dma_start(out=wg[:, 0, :], in_=wb[0][:, 0, :])
                        nc.gpsimd.dma_start(out=xg[:, 0, :NB], in_=xb[0][:, 0, :NB])
                        nc.gpsimd.dma_start(out=xg[:, 0, NB:], in_=xb[0][:, 0, NB:])
                        nc.gpsimd.dma_start(out=xg[:, 1, :], in_=xb[0][:, 1, :])
                        nc.sync.dma_start(out=wg[:, 1, :], in_=wb[0][:, 1, :])
                    else:
                        nc.sync.dma_start(out=xg[:], in_=xb[bidx[g]])
                        nc.gpsimd.dma_start(out=wg[:], in_=wb[bidx[g]])
                og = op.tile([128, RT, BC], BF16, tag="o")
                return {"is8": is8, "xg": xg, "wg": wg, "og": og}

            def mm(st, ps_dst, rt, b0, b1):
                """Accumulate x[:, b0:b1] @ W[rt-tile] into ps_dst."""
                if st["is8"]:
                    nc.tensor.matmul(
                        out=ps_dst,
                        lhsT=st["wg"][:, :, rt * 128:(rt + 1) * 128],
                        rhs=st["xg"][:, :, b0:b1],
                        start=True, stop=True, perf_mode=DR,
                    )
                else:
                    for k in range(KCH):
                        nc.tensor.matmul(
                            out=ps_dst,
                            lhsT=st["wg"][:, k, rt * 128:(rt + 1) * 128],
                            rhs=st["xg"][:, k, b0:b1],
                            start=(k == 0), stop=(k == KCH - 1),
                        )

            def do_tile(g, st, rt):
                ps = pp.tile([128, BH, NB], F32, tag="ps")
                for bh in range(BH):
                    mm(st, ps[:, bh, :], rt, bh * NB, (bh + 1) * NB)
                og = st["og"]
                eng = copy_eng()
                if eng is nc.scalar:
                    eng.copy(out=og[:, rt, :], in_=ps[:])
                else:
                    eng.tensor_copy(out=og[:, rt, :], in_=ps[:])
                if rt % 2 == 1:
                    # half-group store right after the rt1 / rt3 copies
                    h = rt // 2
                    store_eng().dma_start(
                        out=outT2[:, g * RT + 2 * h:g * RT + 2 * h + 2, :],
                        in_=og[:, 2 * h:2 * h + 2, :])

            def do_tail_group(g, st):
                # Hand-scheduled tail group: per-rt stores, per-bh copies for
                # rt2, and rt3's second half split 384+128 so the final
                # copy+store chain carries only 32KB.
                og = st["og"]
                for rt in range(RT - 1):
                    ps = pp.tile([128, BH, NB], F32, tag="ps")
                    for bh in range(BH):
                        mm(st, ps[:, bh, :], rt, bh * NB, (bh + 1) * NB)
                        if rt == RT - 2:
                            dst = og[:, rt, bh * NB:(bh + 1) * NB]
                            if bh == 0:
                                nc.scalar.copy(out=dst, in_=ps[:, bh, :])
                            else:
                                nc.vector.tensor_copy(out=dst, in_=ps[:, bh, :])
                    if rt != RT - 2:
                        eng = copy_eng()
                        if eng is nc.scalar:
                            eng.copy(out=og[:, rt, :], in_=ps[:])
                        else:
                            eng.tensor_copy(out=og[:, rt, :], in_=ps[:])
                    seng = (nc.gpsimd, nc.sync, nc.gpsimd)[rt]
                    seng.dma_start(out=outT2[:, g * RT + rt, :],
                                   in_=og[:, rt, :])
                rt = RT - 1
                ps = pp.tile([128, BH, NB], F32, tag="ps")
                mm(st, ps[:, 0, :], rt, 0, NB)
                nc.scalar.copy(out=og[:, rt, :NB], in_=ps[:, 0, :])
                nc.sync.dma_start(out=outT2[:, g * RT + rt, :NB],
                                  in_=og[:, rt, :NB])
                # rt3's second half as two 256-col pieces, each in its own
                # PSUM bank with its own copy+store on engines that are idle
                # by then; whichever the scheduler runs last, the trailing
                # chain (sem + ~395 copy + 500 store + 1716 flight + barrier)
                # carries only 64KB.
                HP = (BC - NB) // 2
                mm(st, ps[:, 1, :HP], rt, NB, NB + HP)
                nc.vector.tensor_copy(out=og[:, rt, NB:NB + HP],
                                      in_=ps[:, 1, :HP])
                nc.sync.dma_start(
                    out=outT2[:, g * RT + rt, NB:NB + HP],
                    in_=og[:, rt, NB:NB + HP])
                ps2 = pp.tile([128, BH, NB], F32, tag="ps")
                mm(st, ps2[:, 0, :HP], rt, NB + HP, BC)
                nc.scalar.copy(out=og[:, rt, NB + HP:],
                               in_=ps2[:, 0, :HP])
                nc.scalar.dma_start(
                    out=outT2[:, g * RT + rt, NB + HP:],
                    in_=og[:, rt, NB + HP:])

            # Interleave each fp8 group's r-tiles with the following bf16
            # group: the PE produces PSUM tiles 4x faster (sim) during an fp8
            # group than the two copy engines drain them, so alternating
            # bf16/fp8 tiles keeps the 4-deep PSUM rotation from stalling.
            g = 0
            while g < G:
                if g in fidx and g + 1 < G - 1:
                    stf = emit_loads(g)
                    stb = emit_loads(g + 1)
                    for rt in range(RT):
                        do_tile(g + 1, stb, rt)
                        do_tile(g, stf, rt)
                    g += 2
                else:
                    st = emit_loads(g)
                    if g == G - 1:
                        do_tail_group(g, st)
                    else:
                        for rt in range(RT):
                            do_tile(g, st, rt)
                    g += 1
    if not nc.is_finalized():
        nc.finalize()
    return nc


def _get_bass():
    if "nc" not in _BASS_CACHE:
        _BASS_CACHE["nc"] = _build_bass()
    return _BASS_CACHE["nc"]


def _prepare_inputs(x, W, b, perm):
    bf16 = ml_dtypes.bfloat16
    fp8 = ml_dtypes.float8_e4m3
    fset = set(FP8_GROUPS)
    perm_flat = np.asarray(perm).reshape(-1)

    # Gather the permuted channels (within-row gather: cache friendly), cast
    # to bf16, then transpose to channel-major [C, B] = [(g,k,s'), b].
    xg = np.ascontiguousarray(x)[:, perm_flat].astype(bf16)   # [B, C]
    xT = np.ascontiguousarray(xg.T).reshape(G, KCH, 128, B)   # [g, k, s', b]

    # weights: Wt[g][s', k, r] = W[g, r, k*128+s']
    Wt = np.asarray(W).reshape(G, R, KCH, 128).transpose(0, 3, 2, 1)  # [g,s',k,r]
    Wt = np.ascontiguousarray(Wt).astype(bf16)

    bgs = [g for g in range(G) if g not in fset]
    in_maps = []
    wb_all = np.stack([Wt[g] for g in bgs]) if bgs else None
    w8_all = (np.stack([Wt[g].astype(fp8) for g in FP8_GROUPS])
              if FP8_GROUPS else None)
    for c in range(N_CORES):
        sl = slice(c * BC, (c + 1) * BC)
        # [g, k, s', bc] -> per-group [s', k, bc]
        xc = xT[:, :, :, sl].transpose(0, 2, 1, 3)            # [g, s', k, bc]
        m = {
            "xb": np.ascontiguousarray(np.stack([xc[g] for g in bgs])),
            "wb": wb_all,
        }
        if FP8_GROUPS:
            m["x8"] = np.ascontiguousarray(
                np.stack([xc[g] for g in FP8_GROUPS])).astype(fp8)
            m["w8"] = w8_all
        in_maps.append(m)
    return in_maps


def kernel(x, W, b, perm, _trace=False, _trace_kwargs=None):
    nc = _get_bass()
    in_maps = _prepare_inputs(x, W, b, perm)
    res = run_bass_kernel_spmd(
        nc, in_maps, list(range(N_CORES)),
        trace=_trace, **(_trace_kwargs or {}),
    )
    b_flat = np.asarray(b, dtype=np.float32).reshape(-1)
    out = np.empty((B, G * R), dtype=np.float32)
    for c in range(N_CORES):
        blk = res.results[c]["outT2"]                    # [128, G*RT, BC] bf16
        blk = np.ascontiguousarray(blk.transpose(1, 0, 2)).reshape(G * R, BC)
        blk = blk.T.astype(np.float32)                   # [BC, G*R]
        blk += b_flat[None, :]
        out[c * BC:(c + 1) * BC, :] = blk
    if _trace:
        return out, res
    return out


# revision 15
# speedup vs baseline: 1.0191x; 1.0109x over previous
"""GroupedLinear Trainium2 kernel (v2: bf16 + fp8-DoubleRow hybrid).

Math: out[b, g*R + r] = sum_s x[b, perm[g, s]] * W[g, r, s] + bias[g, r]
with B=8192, C=4096, G=16, S=256, R=512.

Strategy (batch-parallel over 8 cores, BC=1024 columns/core):
* Host: apply the channel permutation while building a transposed,
  group-contiguous activation tensor x[(g,k,s'), b] and per-group transposed
  weights W[g][s', k, r]. A subset FP8_GROUPS of the 16 groups is shipped as
  fp8e4m3 (both operands); the rest bf16. Bias is folded in on the host.
* Device, per group:
  - bf16 group: per r-tile rt (4) and batch half bh (2): 2 K=128 bf16
    matmuls accumulate into one half of a 2-bank PSUM tile [128, 2, 512].
  - fp8 group: per (rt, bh): ONE DoubleRow fp8 matmul (K=256: lhsT
    [128,2,128], rhs [128,2,512]) into the same PSUM shape. DoubleRow is 2x
    on the PE; the e4m3 quantization error is rel-L2 3.8e-2 for a full-fp8
    group, so only 4/16 groups run fp8 keeping total rel err ~1.9e-2 < 2e-2.
  - copies: one [128, 1024] fp32->bf16 copy per r-tile (reads both PSUM
    banks), alternating ACT / DVE.
  - loads: one x DMA + one w DMA per group (SP for x, Pool SWDGE for w),
    except group 0 which keeps the split load order that gets the first
    matmul started at ~2.4us (DMA latency floor).
  - stores: one [128, 4, 1024] DMA per group into a transposed output
    outT2[p, g*4+rt, b] (host untransposes), alternating SP/Pool. The last
    group is hand-scheduled: per-bh copies, per-rt stores, and the final
    r-tile's second half is split 384+128 so the very last copy+store
    chain (the unavoidable ~2.7us DMA-drain tail) carries only 32KB.

Notes:
* PE roofline (bf16, all 16 groups) is 54.6us/core; baseline measured
  59.8us. fp8 DoubleRow on 4 groups cuts PE work by 12.5% (hw 2x).
* DMA dispatch time lands on the issuing engine (~0.65-1.0us per DMA), so
  coarse per-group DMAs keep SP/Pool far below the PE time.
* fp8 DoubleRow with full 128-partition output works on hw (verified
  against numpy at rel 1e-4); the earlier "PSUM partitions 0-63" note was
  wrong.
* The PE p-state ramp is wall-clock based (full speed after 3us); the ~2
  matmuls that run before 3us at half clock are unavoidable (DMA latency
  floor for the first operands).
"""

import numpy as np
import ml_dtypes

import concourse.bass as bass
import concourse.mybir as mybir
import concourse.tile as tile
from concourse import bacc
from concourse.bass_utils import run_bass_kernel_spmd

B, C, G, S, R = 8192, 4096, 16, 256, 512
N_CORES = 8
BC = B // N_CORES          # 1024 batch columns per core
KCH = S // 128             # 2 contraction chunks per group
RT = R // 128              # 4 r-tiles per group
NB = 512                   # matmul moving-dim size (one PSUM bank of fp32)
BH = BC // NB              # 2 batch halves per core

# Which logical groups run in fp8 DoubleRow. Keep group 0 (tuned startup
# order) and group 15 (hand-scheduled tail) in bf16. Error budget: rel-L2
# grows as 3.8e-2 * sqrt(n/16); n=4 -> ~1.9e-2 (gate 2e-2).
FP8_GROUPS = (2, 5, 8, 12)

BF16 = mybir.dt.bfloat16
FP8 = mybir.dt.float8e4
F32 = mybir.dt.float32

_BASS_CACHE: dict = {}


def _build_bass():
    n8 = len(FP8_GROUPS)
    nb16 = G - n8
    fidx = {g: i for i, g in enumerate(FP8_GROUPS)}
    bidx = {g: i for i, g in enumerate(g for g in range(G) if g not in fidx)}

    nc = bacc.Bacc(None, num_swdge_queues=4)
    xb = nc.declare_dram_parameter("xb", [nb16, 128, KCH, BC], BF16, isOutput=False)
    wb = nc.declare_dram_parameter("wb", [nb16, 128, KCH, R], BF16, isOutput=False)
    if n8:
        x8 = nc.declare_dram_parameter("x8", [n8, 128, KCH, BC], FP8, isOutput=False)
        w8 = nc.declare_dram_parameter("w8", [n8, 128, KCH, R], FP8, isOutput=False)
    outT2 = nc.declare_dram_parameter("outT2", [128, G * RT, BC], BF16, isOutput=True)

    DR = mybir.MatmulPerfMode.DoubleRow

    # Store-engine schedule for the 30 half-group stores (groups 0-14, two
    # [128, 2, 1024] halves each). CoreSim charges DMA transfer time on the
    # issuing engine (~0.3855 ns per per-partition byte, 500ns floor), so
    # engine busy must be balanced: loads go x->SP, w->Pool; stores fill the
    # remaining SP/Pool capacity only -- a store on ACT/DVE would delay
    # pending PSUM copies, which stalls the PE via the 4-deep PSUM rotation.
    store_share = {"gpsimd": 17, "sync": 13}
    store_seq = []
    used = dict.fromkeys(store_share, 0)
    for i in range(30):
        eng = max(store_share,
                  key=lambda e: store_share[e] * (i + 1) / 30 - used[e])
        used[eng] += 1
        store_seq.append(eng)

    with tile.TileContext(nc) as tc:
        with (
            tc.tile_pool(name="xbp", bufs=4) as xbp,
            tc.tile_pool(name="wbp", bufs=4) as wbp,
            tc.tile_pool(name="x8p", bufs=2) as x8p,
            tc.tile_pool(name="w8p", bufs=2) as w8p,
            tc.tile_pool(name="op", bufs=5) as op,
            tc.tile_pool(name="pp", bufs=4, space="PSUM") as pp,
        ):
            copy_flip = 0
            store_i = 0
            engines = {"sync": nc.sync, "gpsimd": nc.gpsimd,
                       "scalar": nc.scalar}

            def copy_eng():
                nonlocal copy_flip
                copy_flip += 1
                return nc.scalar if copy_flip % 2 else nc.vector

            def store_eng():
                nonlocal store_i
                eng = engines[store_seq[store_i]]
                store_i += 1
                return eng

            def emit_loads(g):
                is8 = g in fidx
                if is8:
                    xg = x8p.tile([128, KCH, BC], FP8, tag="x8")
                    wg = w8p.tile([128, KCH, R], FP8, tag="w8")
                    nc.sync.dma_start(out=xg[:], in_=x8[fidx[g]])
                    nc.gpsimd.dma_start(out=wg[:], in_=w8[fidx[g]])
                else:
                    xg = xbp.tile([128, KCH, BC], BF16, tag="xb")
                    wg = wbp.tile([128, KCH, R], BF16, tag="wb")
                    if g == 0:
                        # tuned startup: the first matmul's operands (w k0 on
                        # SP, x k0 first half on Pool) land as early as
                        # possible
                        nc.sync.dma_start(out=wg[:, 0, :], in_=wb[0][:, 0, :])
                        nc.gpsimd.dma_start(out=xg[:, 0, :NB], in_=xb[0][:, 0, :NB])
                        nc.gpsimd.dma_start(out=xg[:, 0, NB:], in_=xb[0][:, 0, NB:])
                        nc.gpsimd.dma_start(out=xg[:, 1, :], in_=xb[0][:, 1, :])
                        nc.sync.dma_start(out=wg[:, 1, :], in_=wb[0][:, 1, :])
                    else:
                        nc.sync.dma_start(out=xg[:], in_=xb[bidx[g]])
                        nc.gpsimd.dma_start(out=wg[:], in_=wb[bidx[g]])
                og = op.tile([128, RT, BC], BF16, tag="o")
                return {"is8": is8, "xg": xg, "wg": wg, "og": og}

            def mm(st, ps_dst, rt, b0, b1):
                """Accumulate x[:, b0:b1] @ W[rt-tile] into ps_dst."""
                if st["is8"]:
                    nc.tensor.matmul(
                        out=ps_dst,
                        lhsT=st["wg"][:, :, rt * 128:(rt + 1) * 128],
                        rhs=st["xg"][:, :, b0:b1],
                        start=True, stop=True, perf_mode=DR,
                    )
                else:
                    for k in range(KCH):
                        nc.tensor.matmul(
                            out=ps_dst,
                            lhsT=st["wg"][:, k, rt * 128:(rt + 1) * 128],
                            rhs=st["xg"][:, k, b0:b1],
                            start=(k == 0), stop=(k == KCH - 1),
                        )

            def do_tile(g, st, rt):
                ps = pp.tile([128, BH, NB], F32, tag="ps")
                for bh in range(BH):
                    mm(st, ps[:, bh, :], rt, bh * NB, (bh + 1) * NB)
                og = st["og"]
                eng = copy_eng()
                if eng is nc.scalar:
                    eng.copy(out=og[:, rt, :], in_=ps[:])
                else:
                    eng.tensor_copy(out=og[:, rt, :], in_=ps[:])
                if rt % 2 == 1:
                    # half-group store right after the rt1 / rt3 copies
                    h = rt // 2
                    store_eng().dma_start(
                        out=outT2[:, g * RT + 2 * h:g * RT + 2 * h + 2, :],
                        in_=og[:, 2 * h:2 * h + 2, :])

            def do_tail_group(g, st):
                # Hand-scheduled tail group: per-rt stores, per-bh copies for
                # rt2, and rt3's second half split 384+128 so the final
                # copy+store chain carries only 32KB.
                og = st["og"]
                for rt in range(RT - 1):
                    ps = pp.tile([128, BH, NB], F32, tag="ps")
                    for bh in range(BH):
                        mm(st, ps[:, bh, :], rt, bh * NB, (bh + 1) * NB)
                        if rt == RT - 2:
                            dst = og[:, rt, bh * NB:(bh + 1) * NB]
                            if bh == 0:
                                nc.scalar.copy(out=dst, in_=ps[:, bh, :])
                            else:
                                nc.vector.tensor_copy(out=dst, in_=ps[:, bh, :])
                    if rt != RT - 2:
                        eng = copy_eng()
                        if eng is nc.scalar:
                            eng.copy(out=og[:, rt, :], in_=ps[:])
                        else:
                            eng.tensor_copy(out=og[:, rt, :], in_=ps[:])
                    seng = (nc.gpsimd, nc.sync, nc.gpsimd)[rt]
                    seng.dma_start(out=outT2[:, g * RT + rt, :],
                                   in_=og[:, rt, :])
                rt = RT - 1
                ps = pp.tile([128, BH, NB], F32, tag="ps")
                mm(st, ps[:, 0, :], rt, 0, NB)
                nc.scalar.copy(out=og[:, rt, :NB], in_=ps[:, 0, :])
                nc.sync.dma_start(out=outT2[:, g * RT + rt, :NB],
                                  in_=og[:, rt, :NB])
                # rt3's second half as two 256-col pieces, each in its own
                # PSUM bank with its own copy+store on engines that are idle
                # by then; whichever the scheduler runs last, the trailing
                # chain (sem + ~395 copy + 500 store + 1716 flight + barrier)
                # carries only 64KB.
                HP = (BC - NB) // 2
                mm(st, ps[:, 1, :HP], rt, NB, NB + HP)
                nc.vector.tensor_copy(out=og[:, rt, NB:NB + HP],
                                      in_=ps[:, 1, :HP])
                nc.sync.dma_start(
                    out=outT2[:, g * RT + rt, NB:NB + HP],
                    in_=og[:, rt, NB:NB + HP])
                ps2 = pp.tile([128, BH, NB], F32, tag="ps")
                mm(st, ps2[:, 0, :HP], rt, NB + HP, BC)
                nc.scalar.copy(out=og[:, rt, NB + HP:],
                               in_=ps2[:, 0, :HP])
                nc.scalar.dma_start(
                    out=outT2[:, g * RT + rt, NB + HP:],
                    in_=og[:, rt, NB + HP:])

            # Interleave each fp8 group's r-tiles with the following bf16
            # group: the PE produces PSUM tiles 4x faster (sim) during an fp8
            # group than the two copy engines drain them, so alternating
            # bf16/fp8 tiles keeps the 4-deep PSUM rotation from stalling.
            g = 0
            while g < G:
                if g in fidx and g + 1 < G - 1:
                    stf = emit_loads(g)
                    stb = emit_loads(g + 1)
                    for rt in range(RT):
                        do_tile(g + 1, stb, rt)
                        do_tile(g, stf, rt)
                    g += 2
                else:
                    st = emit_loads(g)
                    if g == G - 1:
                        do_tail_group(g, st)
                    else:
                        for rt in range(RT):
                            do_tile(g, st, rt)
                    g += 1
    if not nc.is_finalized():
        nc.finalize()
    return nc


def _get_bass():
    if "nc" not in _BASS_CACHE:
        _BASS_CACHE["nc"] = _build_bass()
    return _BASS_CACHE["nc"]


def _prepare_inputs(x, W, b, perm):
    bf16 = ml_dtypes.bfloat16
    fp8 = ml_dtypes.float8_e4m3
    fset = set(FP8_GROUPS)
    perm_flat = np.asarray(perm).reshape(-1)

    # Gather the permuted channels (within-row gather: cache friendly), cast
    # to bf16, then transpose to channel-major [C, B] = [(g,k,s'), b].
    xg = np.ascontiguousarray(x)[:, perm_flat].astype(bf16)   # [B, C]
    xT = np.ascontiguousarray(xg.T).reshape(G, KCH, 128, B)   # [g, k, s', b]

    # weights: Wt[g][s', k, r] = W[g, r, k*128+s']
    Wt = np.asarray(W).reshape(G, R, KCH, 128).transpose(0, 3, 2, 1)  # [g,s',k,r]
    Wt = np.ascontiguousarray(Wt).astype(bf16)

    bgs = [g for g in range(G) if g not in fset]
    in_maps = []
    wb_all = np.stack([Wt[g] for g in bgs]) if bgs else None
    w8_all = (np.stack([Wt[g].astype(fp8) for g in FP8_GROUPS])
              if FP8_GROUPS else None)
    for c in range(N_CORES):
        sl = slice(c * BC, (c + 1) * BC)
        # [g, k, s', bc] -> per-group [s', k, bc]
        xc = xT[:, :, :, sl].transpose(0, 2, 1, 3)            # [g, s', k, bc]
        m = {
            "xb": np.ascontiguousarray(np.stack([xc[g] for g in bgs])),
            "wb": wb_all,
        }
        if FP8_GROUPS:
            m["x8"] = np.ascontiguousarray(
                np.stack([xc[g] for g in FP8_GROUPS])).astype(fp8)
            m["w8"] = w8_all
        in_maps.append(m)
    return in_maps


def kernel(x, W, b, perm, _trace=False, _trace_kwargs=None):
    nc = _get_bass()
    in_maps = _prepare_inputs(x, W, b, perm)
    res = run_bass_kernel_spmd(
        nc, in_maps, list(range(N_CORES)),
        trace=_trace, **(_trace_kwargs or {}),
    )
    b_flat = np.asarray(b, dtype=np.float32).reshape(-1)
    out = np.empty((B, G * R), dtype=np.float32)
    for c in range(N_CORES):
        blk = res.results[c]["outT2"]                    # [128, G*RT, BC] bf16
        blk = np.ascontiguousarray(blk.transpose(1, 0, 2)).reshape(G * R, BC)
        blk = blk.T.astype(np.float32)                   # [BC, G*R]
        blk += b_flat[None, :]
        out[c * BC:(c + 1) * BC, :] = blk
    if _trace:
        return out, res
    return out


# revision 16
# speedup vs baseline: 1.0200x; 1.0009x over previous
"""GroupedLinear Trainium2 kernel (v2: bf16 + fp8-DoubleRow hybrid).

Math: out[b, g*R + r] = sum_s x[b, perm[g, s]] * W[g, r, s] + bias[g, r]
with B=8192, C=4096, G=16, S=256, R=512.

Strategy (batch-parallel over 8 cores, BC=1024 columns/core):
* Host: apply the channel permutation while building a transposed,
  group-contiguous activation tensor x[(g,k,s'), b] and per-group transposed
  weights W[g][s', k, r]. A subset FP8_GROUPS of the 16 groups is shipped as
  fp8e4m3 (both operands); the rest bf16. Bias is folded in on the host.
* Device, per group:
  - bf16 group: per r-tile rt (4) and batch half bh (2): 2 K=128 bf16
    matmuls accumulate into one half of a 2-bank PSUM tile [128, 2, 512].
  - fp8 group: per (rt, bh): ONE DoubleRow fp8 matmul (K=256: lhsT
    [128,2,128], rhs [128,2,512]) into the same PSUM shape. DoubleRow is 2x
    on the PE; the e4m3 quantization error is rel-L2 3.8e-2 for a full-fp8
    group, so only 4/16 groups run fp8 keeping total rel err ~1.9e-2 < 2e-2.
  - copies: one [128, 1024] fp32->bf16 copy per r-tile (reads both PSUM
    banks), alternating ACT / DVE.
  - loads: one x DMA + one w DMA per group (SP for x, Pool SWDGE for w),
    except group 0 which keeps the split load order that gets the first
    matmul started at ~2.4us (DMA latency floor).
  - stores: one [128, 4, 1024] DMA per group into a transposed output
    outT2[p, g*4+rt, b] (host untransposes), alternating SP/Pool. The last
    group is hand-scheduled: per-bh copies, per-rt stores, and the final
    r-tile's second half is split 384+128 so the very last copy+store
    chain (the unavoidable ~2.7us DMA-drain tail) carries only 32KB.

Notes:
* PE roofline (bf16, all 16 groups) is 54.6us/core; baseline measured
  59.8us. fp8 DoubleRow on 4 groups cuts PE work by 12.5% (hw 2x).
* DMA dispatch time lands on the issuing engine (~0.65-1.0us per DMA), so
  coarse per-group DMAs keep SP/Pool far below the PE time.
* fp8 DoubleRow with full 128-partition output works on hw (verified
  against numpy at rel 1e-4); the earlier "PSUM partitions 0-63" note was
  wrong.
* The PE p-state ramp is wall-clock based (full speed after 3us); the ~2
  matmuls that run before 3us at half clock are unavoidable (DMA latency
  floor for the first operands).
"""

import numpy as np
import ml_dtypes

import concourse.bass as bass
import concourse.mybir as mybir
import concourse.tile as tile
from concourse import bacc
from concourse.bass_utils import run_bass_kernel_spmd

B, C, G, S, R = 8192, 4096, 16, 256, 512
N_CORES = 8
BC = B // N_CORES          # 1024 batch columns per core
KCH = S // 128             # 2 contraction chunks per group
RT = R // 128              # 4 r-tiles per group
NB = 512                   # matmul moving-dim size (one PSUM bank of fp32)
BH = BC // NB              # 2 batch halves per core

# Which logical groups run in fp8 DoubleRow. Keep group 0 (tuned startup
# order) and group 15 (hand-scheduled tail) in bf16. Error budget: rel-L2
# grows as 3.8e-2 * sqrt(n/16); n=4 -> ~1.9e-2 (gate 2e-2).
FP8_GROUPS = (2, 5, 8, 12)

BF16 = mybir.dt.bfloat16
FP8 = mybir.dt.float8e4
F32 = mybir.dt.float32

_BASS_CACHE: dict = {}


def _build_bass():
    n8 = len(FP8_GROUPS)
    nb16 = G - n8
    fidx = {g: i for i, g in enumerate(FP8_GROUPS)}
    bidx = {g: i for i, g in enumerate(g for g in range(G) if g not in fidx)}

    nc = bacc.Bacc(None, num_swdge_queues=4)
    xb = nc.declare_dram_parameter("xb", [nb16, 128, KCH, BC], BF16, isOutput=False)
    wb = nc.declare_dram_parameter("wb", [nb16, 128, KCH, R], BF16, isOutput=False)
    if n8:
        x8 = nc.declare_dram_parameter("x8", [n8, 128, KCH, BC], FP8, isOutput=False)
        w8 = nc.declare_dram_parameter("w8", [n8, 128, KCH, R], FP8, isOutput=False)
    outT2 = nc.declare_dram_parameter("outT2", [128, G * RT, BC], BF16, isOutput=True)

    DR = mybir.MatmulPerfMode.DoubleRow

    # Store-engine schedule for the 30 half-group stores (groups 0-14, two
    # [128, 2, 1024] halves each). CoreSim charges DMA transfer time on the
    # issuing engine (~0.3855 ns per per-partition byte, 500ns floor), so
    # engine busy must be balanced: loads go x->SP, w->Pool; stores fill the
    # remaining SP/Pool capacity only -- a store on ACT/DVE would delay
    # pending PSUM copies, which stalls the PE via the 4-deep PSUM rotation.
    store_share = {"gpsimd": 17, "sync": 13}
    store_seq = []
    used = dict.fromkeys(store_share, 0)
    for i in range(30):
        eng = max(store_share,
                  key=lambda e: store_share[e] * (i + 1) / 30 - used[e])
        used[eng] += 1
        store_seq.append(eng)

    with tile.TileContext(nc) as tc:
        with (
            tc.tile_pool(name="xbp", bufs=4) as xbp,
            tc.tile_pool(name="wbp", bufs=4) as wbp,
            tc.tile_pool(name="x8p", bufs=2) as x8p,
            tc.tile_pool(name="w8p", bufs=2) as w8p,
            tc.tile_pool(name="op", bufs=5) as op,
            tc.tile_pool(name="pp", bufs=4, space="PSUM") as pp,
        ):
            copy_flip = 0
            store_i = 0
            engines = {"sync": nc.sync, "gpsimd": nc.gpsimd,
                       "scalar": nc.scalar}

            def copy_eng():
                nonlocal copy_flip
                copy_flip += 1
                return nc.scalar if copy_flip % 2 else nc.vector

            def store_eng():
                nonlocal store_i
                eng = engines[store_seq[store_i]]
                store_i += 1
                return eng

            def emit_loads(g):
                is8 = g in fidx
                if is8:
                    xg = x8p.tile([128, KCH, BC], FP8, tag="x8")
                    wg = w8p.tile([128, KCH, R], FP8, tag="w8")
                    nc.sync.dma_start(out=xg[:], in_=x8[fidx[g]])
                    nc.gpsimd.dma_start(out=wg[:], in_=w8[fidx[g]])
                else:
                    xg = xbp.tile([128, KCH, BC], BF16, tag="xb")
                    wg = wbp.tile([128, KCH, R], BF16, tag="wb")
                    if g == 0:
                        # tuned startup: the first matmul's operands (w k0 on
                        # SP, x k0 first half on Pool) land as early as
                        # possible
                        nc.sync.dma_start(out=xg[:, 0, :NB], in_=xb[0][:, 0, :NB])
                        nc.gpsimd.dma_start(out=wg[:, 0, :], in_=wb[0][:, 0, :])
                        nc.gpsimd.dma_start(out=xg[:, 0, NB:], in_=xb[0][:, 0, NB:])
                        nc.sync.dma_start(out=xg[:, 1, :], in_=xb[0][:, 1, :])
                        nc.gpsimd.dma_start(out=wg[:, 1, :], in_=wb[0][:, 1, :])
                    else:
                        nc.sync.dma_start(out=xg[:], in_=xb[bidx[g]])
                        nc.gpsimd.dma_start(out=wg[:], in_=wb[bidx[g]])
                og = op.tile([128, RT, BC], BF16, tag="o")
                return {"is8": is8, "xg": xg, "wg": wg, "og": og}

            def mm(st, ps_dst, rt, b0, b1):
                """Accumulate x[:, b0:b1] @ W[rt-tile] into ps_dst."""
                if st["is8"]:
                    nc.tensor.matmul(
                        out=ps_dst,
                        lhsT=st["wg"][:, :, rt * 128:(rt + 1) * 128],
                        rhs=st["xg"][:, :, b0:b1],
                        start=True, stop=True, perf_mode=DR,
                    )
                else:
                    for k in range(KCH):
                        nc.tensor.matmul(
                            out=ps_dst,
                            lhsT=st["wg"][:, k, rt * 128:(rt + 1) * 128],
                            rhs=st["xg"][:, k, b0:b1],
                            start=(k == 0), stop=(k == KCH - 1),
                        )

            def do_tile(g, st, rt):
                ps = pp.tile([128, BH, NB], F32, tag="ps")
                for bh in range(BH):
                    mm(st, ps[:, bh, :], rt, bh * NB, (bh + 1) * NB)
                og = st["og"]
                eng = copy_eng()
                if eng is nc.scalar:
                    eng.copy(out=og[:, rt, :], in_=ps[:])
                else:
                    eng.tensor_copy(out=og[:, rt, :], in_=ps[:])
                if rt % 2 == 1:
                    # half-group store right after the rt1 / rt3 copies
                    h = rt // 2
                    store_eng().dma_start(
                        out=outT2[:, g * RT + 2 * h:g * RT + 2 * h + 2, :],
                        in_=og[:, 2 * h:2 * h + 2, :])

            def do_tail_group(g, st):
                # Hand-scheduled tail group: per-rt stores, per-bh copies for
                # rt2, and rt3's second half split 384+128 so the final
                # copy+store chain carries only 32KB.
                og = st["og"]
                for rt in range(RT - 1):
                    ps = pp.tile([128, BH, NB], F32, tag="ps")
                    for bh in range(BH):
                        mm(st, ps[:, bh, :], rt, bh * NB, (bh + 1) * NB)
                        if rt == RT - 2:
                            dst = og[:, rt, bh * NB:(bh + 1) * NB]
                            if bh == 0:
                                nc.scalar.copy(out=dst, in_=ps[:, bh, :])
                            else:
                                nc.vector.tensor_copy(out=dst, in_=ps[:, bh, :])
                    if rt != RT - 2:
                        eng = copy_eng()
                        if eng is nc.scalar:
                            eng.copy(out=og[:, rt, :], in_=ps[:])
                        else:
                            eng.tensor_copy(out=og[:, rt, :], in_=ps[:])
                    seng = (nc.gpsimd, nc.sync, nc.gpsimd)[rt]
                    seng.dma_start(out=outT2[:, g * RT + rt, :],
                                   in_=og[:, rt, :])
                rt = RT - 1
                ps = pp.tile([128, BH, NB], F32, tag="ps")
                mm(st, ps[:, 0, :], rt, 0, NB)
                nc.scalar.copy(out=og[:, rt, :NB], in_=ps[:, 0, :])
                nc.sync.dma_start(out=outT2[:, g * RT + rt, :NB],
                                  in_=og[:, rt, :NB])
                # rt3's second half as two 256-col pieces, each in its own
                # PSUM bank with its own copy+store on engines that are idle
                # by then; whichever the scheduler runs last, the trailing
                # chain (sem + ~395 copy + 500 store + 1716 flight + barrier)
                # carries only 64KB.
                HP = (BC - NB) // 2
                mm(st, ps[:, 1, :HP], rt, NB, NB + HP)
                nc.vector.tensor_copy(out=og[:, rt, NB:NB + HP],
                                      in_=ps[:, 1, :HP])
                nc.sync.dma_start(
                    out=outT2[:, g * RT + rt, NB:NB + HP],
                    in_=og[:, rt, NB:NB + HP])
                ps2 = pp.tile([128, BH, NB], F32, tag="ps")
                mm(st, ps2[:, 0, :HP], rt, NB + HP, BC)
                nc.scalar.copy(out=og[:, rt, NB + HP:],
                               in_=ps2[:, 0, :HP])
                nc.scalar.dma_start(
                    out=outT2[:, g * RT + rt, NB + HP:],
                    in_=og[:, rt, NB + HP:])

            # Interleave each fp8 group's r-tiles with the following bf16
            # group: the PE produces PSUM tiles 4x faster (sim) during an fp8
            # group than the two copy engines drain them, so alternating
            # bf16/fp8 tiles keeps the 4-deep PSUM rotation from stalling.
            g = 0
            while g < G:
                if g in fidx and g + 1 < G - 1:
                    stf = emit_loads(g)
                    stb = emit_loads(g + 1)
                    for rt in range(RT):
                        do_tile(g + 1, stb, rt)
                        do_tile(g, stf, rt)
                    g += 2
                else:
                    st = emit_loads(g)
                    if g == G - 1:
                        do_tail_group(g, st)
                    else:
                        for rt in range(RT):
                            do_tile(g, st, rt)
                    g += 1
    if not nc.is_finalized():
        nc.finalize()
    return nc


def _get_bass():
    if "nc" not in _BASS_CACHE:
        _BASS_CACHE["nc"] = _build_bass()
    return _BASS_CACHE["nc"]


def _prepare_inputs(x, W, b, perm):
    bf16 = ml_dtypes.bfloat16
    fp8 = ml_dtypes.float8_e4m3
    fset = set(FP8_GROUPS)
    perm_flat = np.asarray(perm).reshape(-1)

    # Gather the permuted channels (within-row gather: cache friendly), cast
    # to bf16, then transpose to channel-major [C, B] = [(g,k,s'), b].
    xg = np.ascontiguousarray(x)[:, perm_flat].astype(bf16)   # [B, C]
    xT = np.ascontiguousarray(xg.T).reshape(G, KCH, 128, B)   # [g, k, s', b]

    # weights: Wt[g][s', k, r] = W[g, r, k*128+s']
    Wt = np.asarray(W).reshape(G, R, KCH, 128).transpose(0, 3, 2, 1)  # [g,s',k,r]
    Wt = np.ascontiguousarray(Wt).astype(bf16)

    bgs = [g for g in range(G) if g not in fset]
    in_maps = []
    wb_all = np.stack([Wt[g] for g in bgs]) if bgs else None
    w8_all = (np.stack([Wt[g].astype(fp8) for g in FP8_GROUPS])
              if FP8_GROUPS else None)
    for c in range(N_CORES):
        sl = slice(c * BC, (c + 1) * BC)
        # [g, k, s', bc] -> per-group [s', k, bc]
        xc = xT[:, :, :, sl].transpose(0, 2, 1, 3)            # [g, s', k, bc]
        m = {
            "xb": np.ascontiguousarray(np.stack([xc[g] for g in bgs])),
            "wb": wb_all,
        }
        if FP8_GROUPS:
            m["x8"] = np.ascontiguousarray(
                np.stack([xc[g] for g in FP8_GROUPS])).astype(fp8)
            m["w8"] = w8_all
        in_maps.append(m)
    return in_maps


def kernel(x, W, b, perm, _trace=False, _trace_kwargs=None):
    nc = _get_bass()
    in_maps = _prepare_inputs(x, W, b, perm)
    res = run_bass_kernel_spmd(
        nc, in_maps, list(range(N_CORES)),
        trace=_trace, **(_trace_kwargs or {}),
    )
    b_flat = np.asarray(b, dtype=np.float32).reshape(-1)
    out = np.empty((B, G * R), dtype=np.float32)
    for c in range(N_CORES):
        blk = res.results[c]["outT2"]                    # [128, G*RT, BC] bf16
        blk = np.ascontiguousarray(blk.transpose(1, 0, 2)).reshape(G * R, BC)
        blk = blk.T.astype(np.float32)                   # [BC, G*R]
        blk += b_flat[None, :]
        out[c * BC:(c + 1) * BC, :] = blk
    if _trace:
        return out, res
    return out
